# revision 13
# baseline (speedup 1.0000x reference)
"""Distributed multi-head attention for trn2 (8 NeuronCores).

Problem: B=4, S=1024, H=1024, nh=16, hd=64; mask is all-ones, biases are
zero (both fixed by the problem's input spec), so neither reaches the
device.

Sharding: core c = b*2 + g handles batch b = c//2 and head-group
g = c%2 (8 heads = 512 hidden dims).  Each core computes
  qT,kT = (Wq_g @ x_b.T), (Wk_g @ x_b.T)      [512, 1024]
  v     = x_b @ Wv_g.T                          [1024, 512]
  per head: scoresT = kT_h.T-contract-qT_h      [tk, tq] psum
            probsT  = exp(scoresT / 8)          (ACT, fp8 out)
            ctxT_aug = [v_h | 1].T @ probsT     rows 0-63 ctx, row 64+ = rowsum
            ctxT = ctxT_aug[0:64] * (1/rowsum)  (DVE)
  partial_out = ctxT.T @ Wo_g_rows.T            [1024, 1024]
Host sums the two partials of each batch (row-parallel Wo unshard) and
stacks the 4 batches.

Schedule (v4): one interleaved PE stream.  The attention inner loop is
paced by ACT exp (~1147 ns per [128,1024] chunk) while its own PE work
is ~854 ns/slot, so all other matmul units ride the scores PSUM ring as
compact fillers: QK projections of pair m during heads 2m-2/2m-1, the V
projection in heads 0-1, and the first half of the out-projection
(cc0+cc1 -> bf16 partial) in heads 6-7.  The tail does the last head's
ctx (riding 2 slots behind its own exps), split fins, then per-column
units (cc2+cc3 [+ partial via identity matmul], evacuated alternately
by ACT copy and DVE add, DMA'd alternately via SP and GPSIMD queues).
PSUM: tag "s" bufs=2 + tag "c" bufs=2 = 8 banks.
"""

import sys

import numpy as np

sys.path.insert(0, "/opt/trn_rl_repo")

import ml_dtypes  # noqa: E402

import concourse.bass as bass  # noqa: E402
import concourse.tile as tile  # noqa: E402
from concourse import bacc, mybir  # noqa: E402
from concourse.bass_utils import run_bass_kernel_spmd  # noqa: E402

S = 1024  # sequence length
H = 1024  # hidden
NH_LOC = 8  # heads per core
HD = 64  # head dim
HG = 512  # hidden dims per core's head group
P = 128  # partitions

F32 = mybir.dt.float32
BF16 = mybir.dt.bfloat16
FP8E3 = mybir.dt.float8e3  # e3m4: 4 mantissa bits, ~0.9% probs quant
INPUT_DT = BF16

_CACHE: dict = {}


def _build_graph(reps: int = 1, timing: bool = False, phases=("qkv", "attn", "exp", "ctx", "out")):
    nc = bacc.Bacc(
        "TRN2", target_bir_lowering=False, debug=False, num_devices=8
    )

    kind = "Internal" if timing else "ExternalInput"
    okind = "Internal" if timing else "ExternalOutput"
    xt_d = nc.dram_tensor("xt", [H, S], INPUT_DT, kind=kind).ap()
    wqt_d = nc.dram_tensor("wqt", [H, HG], INPUT_DT, kind=kind).ap()
    wkt_d = nc.dram_tensor("wkt", [H, HG], INPUT_DT, kind=kind).ap()
    wvt_d = nc.dram_tensor("wvt", [H, HG], INPUT_DT, kind=kind).ap()
    wot_d = nc.dram_tensor("wot", [HG, H], BF16, kind=kind).ap()
    id_d = nc.dram_tensor("ident", [P, P], BF16, kind=kind).ap()
    out_d = nc.dram_tensor("out_p", [S, H], F32, kind=okind).ap()
    tok_d = (
        nc.dram_tensor("tok", [1, 4], F32, kind="ExternalOutput").ap()
        if timing
        else None
    )

    with tile.TileContext(nc) as tc:
        with tc.tile_pool(name="inp", bufs=1) as inp:
            tiles = _dma_inputs(tc, inp, xt_d, wqt_d, wkt_d, wvt_d, wot_d, id_d)
            pre = _alloc_persistent(tc, inp)
            if reps == 1:
                _compute(tc, tiles, out_d, tok_d, phases, pre)
            else:
                hints = (
                    mybir.EngineType.PE,
                    mybir.EngineType.DVE,
                    mybir.EngineType.Pool,
                )
                with tc.For_i(0, reps, 1, hint_engines=hints):
                    _compute(tc, tiles, out_d, tok_d, phases, pre)

    nc.compile()
    return nc


def _dma_inputs(tc, inp, xt_d, wqt_d, wkt_d, wvt_d, wot_d, id_d):
    # interleave per contraction-chunk so the first qk accumulation
    # chains can start as soon as chunk 0 lands
    nc = tc.nc
    xt, wqt, wkt, wvt = [], [], [], []
    for kc in range(8):
        t = inp.tile([P, S], INPUT_DT, tag=f"xt{kc}", name=f"xt{kc}")
        nc.sync.dma_start(t[:], xt_d[kc * P : (kc + 1) * P, :])
        xt.append(t)
        for lst, d, tag in (
            (wqt, wqt_d, "wqt"), (wkt, wkt_d, "wkt"), (wvt, wvt_d, "wvt")
        ):
            t = inp.tile([P, HG], INPUT_DT, tag=f"{tag}{kc}", name=f"{tag}{kc}")
            nc.sync.dma_start(t[:], d[kc * P : (kc + 1) * P, :])
            lst.append(t)
    wot = []  # 4 x [128, 1024] bf16, rows = local c
    for cc in range(4):
        t = inp.tile([P, H], BF16, tag=f"wot{cc}", name=f"wot{cc}")
        nc.sync.dma_start(t[:], wot_d[cc * P : (cc + 1) * P, :])
        wot.append(t)
    ident = inp.tile([P, P], BF16, tag="ident", name="ident")
    nc.sync.dma_start(ident[:], id_d[:])
    return xt, wqt, wkt, wvt, wot, ident


def _alloc_persistent(tc, inp):
    """Persistent activation tiles + one-time constant fills (outside any
    For_i): rows 64-127 of qT/kT stay zero (K=128-padded scores); the
    ones columns of v_sb are never overwritten by the per-iteration
    copies, so both are filled once instead of every iteration."""
    nc = tc.nc
    pre = {}
    for h in range(NH_LOC):
        for tag in ("qT", "kT"):
            t = inp.tile([P, S], BF16, tag=f"{tag}{h}", name=f"{tag}{h}")
            nc.gpsimd.memset(t[64:128, :], 0.0)
            pre[f"{tag}{h}"] = t
    for i in range(8):
        # [key-in-chunk, head, 64 v-dims + 64 ones]
        t = inp.tile([P, 8, 128], BF16, tag=f"v{i}", name=f"v{i}")
        nc.gpsimd.memset(t[:, :, HD:128], 1.0)
        pre[f"v{i}"] = t
    for m in range(4):
        t = inp.tile([P, S], BF16, tag=f"ctxT{m}", name=f"ctxT{m}")
        pre[f"ctxT{m}"] = t
    return pre


def _compute(tc, tiles, out_d, tok_d=None, phases=("qkv", "attn", "exp", "ctx", "out"), pre=None):
    pre = pre or {}
    nc = tc.nc
    xt, wqt, wkt, wvt, wot, ident = tiles
    from contextlib import ExitStack

    do_qkv = "qkv" in phases
    do_attn = "attn" in phases
    do_exp = "exp" in phases
    do_ctx = "ctx" in phases
    do_out = "out" in phases

    ctx = ExitStack()
    with ctx:
        probs_pool = ctx.enter_context(tc.tile_pool(name="probs", bufs=18))
        small = ctx.enter_context(tc.tile_pool(name="small", bufs=3))
        outsb = ctx.enter_context(tc.tile_pool(name="outsb", bufs=1))
        # PSUM: tag "s" 3x[128,1024]f32 (6 banks) for scores/exp and every
        # compact filler unit; tag "c" 1x (2 banks) for the ctx chains
        psp = ctx.enter_context(tc.tile_pool(name="psp", bufs=3, space="PSUM"))

        def ps_s(name):
            return psp.tile([P, S], F32, tag="s", bufs=3, name=name)

        def ps_c(name):
            return psp.tile([P, S], F32, tag="c", bufs=1, name=name)

        def mm2(ps, lhsT, rhs, start, stop):
            # ISA limit: one matmul writes at most 512 f32 psum columns
            for th in range(2):
                nc.tensor.matmul(
                    ps[:, th * 512 : (th + 1) * 512],
                    lhsT,
                    rhs[:, th * 512 : (th + 1) * 512],
                    start=start,
                    stop=stop,
                )


        qT = [pre[f"qT{h}"] for h in range(NH_LOC)]
        kT = [pre[f"kT{h}"] for h in range(NH_LOC)]
        v_sb = [pre[f"v{i}"] for i in range(8)]
        ctxT = [pre[f"ctxT{m}"] for m in range(4)]
        partial = [
            outsb.tile([P, H], BF16, tag=f"pt{i}", name=f"pt{i}") for i in range(8)
        ]
        out_acc = [
            outsb.tile([P, H], F32, tag=f"oa{i}", name=f"oa{i}") for i in range(8)
        ]

        # ---- filler units (each rides the "s" PSUM ring, compact) -------
        def unit_qk(which, m):
            if not do_qkv:
                return
            ps = ps_s("ps_qk")
            w = wqt if which == "q" else wkt
            dst = qT if which == "q" else kT
            for kc in range(8):
                mm2(ps, w[kc][:, m * P : (m + 1) * P], xt[kc],
                    start=(kc == 0), stop=(kc == 7))
            # evacuate both head-halves in parallel: DVE + ACT copy so the
            # psum slot frees in ~1.2us instead of 2.4us of serial DVE
            nc.vector.tensor_copy(dst[2 * m][0:64, :], ps[0:64, :])
            nc.scalar.activation(
                dst[2 * m + 1][0:64, :], ps[64:128, :],
                mybir.ActivationFunctionType.Copy,
            )

        def unit_v(tk):
            if not do_qkv:
                return
            ps = psp.tile([P, 8, HD], F32, tag="s", bufs=3, name="ps_v")
            for kc in range(8):
                nc.tensor.matmul(
                    ps[:],
                    xt[kc][:, tk * P : (tk + 1) * P],
                    wvt[kc][:],
                    start=(kc == 0),
                    stop=(kc == 7),
                )
            nc.vector.tensor_copy(v_sb[tk][:, :, 0:HD], ps[:])

        def unit_out_early(tc_i):
            if not do_out:
                return
            ps = ps_s("ps_oe")
            for cc in (0, 1):
                mm2(ps, ctxT[cc][:, tc_i * P : (tc_i + 1) * P], wot[cc],
                    start=(cc == 0), stop=(cc == 1))
            nc.vector.tensor_copy(partial[tc_i][:], ps[:])

        def unit_out_late(tc_i):
            if not do_out:
                return
            ps = ps_s("ps_ol")
            use_act = tc_i % 2 == 0
            mm2(ps, ctxT[2][:, tc_i * P : (tc_i + 1) * P], wot[2],
                start=True, stop=False)
            mm2(ps, ctxT[3][:, tc_i * P : (tc_i + 1) * P], wot[3],
                start=False, stop=not use_act)
            if use_act:
                # fold the bf16 partial into psum via identity matmul, then
                # ACT evacuates; keeps DVE free for the odd units' adds
                mm2(ps, ident[:], partial[tc_i], start=False, stop=True)
                nc.scalar.activation(
                    out_acc[tc_i][:], ps[:],
                    mybir.ActivationFunctionType.Copy,
                )
                eng = nc.sync
            else:
                nc.vector.tensor_tensor(
                    out_acc[tc_i][:], ps[:], partial[tc_i][:],
                    mybir.AluOpType.add,
                )
                eng = nc.gpsimd
            eng.dma_start(
                out_d[tc_i * P : (tc_i + 1) * P, :], out_acc[tc_i][:]
            )

        def fin_part(h, pchain, j, nhalf):
            if not do_ctx:
                return
            m, hh = h // 2, h % 2
            w = S // nhalf
            sl = slice(j * w, (j + 1) * w)
            rp = small.tile([HD, w], F32, tag="rp", name="rp")
            nc.vector.reciprocal(rp[:], pchain[64:128, sl])
            nc.vector.tensor_tensor(
                ctxT[m][hh * HD : (hh + 1) * HD, sl],
                pchain[0:HD, sl],
                rp[:],
                mybir.AluOpType.mult,
            )

        def fin(h, pchain):
            fin_part(h, pchain, 0, 1)

        # filler placement: (head, slot) -> unit thunks.  One compact
        # unit per ~2 slots in the low-PE half (slots 0-3) of each head.
        # Deadlines: v pair (2j,2j+1) before ctx(0) slot 4+j in head 1;
        # qk pair m before head 2m slot 0; out-early after fin(3) (head
        # 4/5 boundary).
        fillers = {}

        def put(h, s, u):
            fillers.setdefault((h, s), []).append(u)

        put(0, 0, lambda: unit_qk("q", 1))
        put(0, 2, lambda: unit_v(0))
        put(0, 4, lambda: unit_v(1))
        put(0, 6, lambda: unit_v(2))
        put(1, 0, lambda: unit_qk("k", 1))
        put(1, 1, lambda: unit_v(3))
        put(1, 2, lambda: unit_v(4))
        put(1, 3, lambda: unit_v(5))
        put(1, 4, lambda: unit_v(6))
        put(1, 5, lambda: unit_v(7))
        put(2, 0, lambda: unit_qk("q", 2))
        put(3, 0, lambda: unit_qk("k", 2))
        put(4, 0, lambda: unit_qk("q", 3))
        put(5, 0, lambda: unit_qk("k", 3))
        for i in range(4):
            put(6, i, lambda i=i: unit_out_early(i))
            put(7, i, lambda i=i: unit_out_early(4 + i))

        # ---- prologue: pair 0 projections -------------------------------
        unit_qk("q", 0)
        unit_qk("k", 0)

        # ---- attention loop --------------------------------------------
        # ctx chain of head h-1 runs compressed in head h slots 4-7 (two
        # accumulation matmuls per slot; all its probs are ready), fin
        # right after -> the single "c" psum tile frees early in head h+1,
        # well before the next chain allocates at head h+1 slot 4.
        prev_probs = None
        for h in range(NH_LOC):
            if not do_attn:
                break
            newchain = None
            last = h == NH_LOC - 1
            probs = []
            for tk in range(8):
                ps = ps_s("ps_s")
                mm2(ps, kT[h][:, tk * P : (tk + 1) * P], qT[h],
                    start=True, stop=True)
                pb = probs_pool.tile([P, S], FP8E3, tag="pb", name="pb")
                if do_exp:
                    nc.scalar.activation(
                        pb[:], ps[:], mybir.ActivationFunctionType.Exp,
                        scale=0.125,
                    )
                probs.append(pb)
                if do_ctx and prev_probs is not None and tk >= 4:
                    if newchain is None:
                        newchain = ps_c("ps_c")
                    for j in (0, 1):
                        ck = 2 * (tk - 4) + j
                        mm2(newchain, v_sb[ck][:, h - 1, :], prev_probs[ck],
                            start=(ck == 0), stop=(ck == 7))
                for u in fillers.get((h, tk), []):
                    u()
            if newchain is not None and not last:
                fin(h - 1, newchain)
            prev_probs = probs
            if last:
                lastfin = newchain  # chain of head 6, finned in the tail

        # ---- tail -------------------------------------------------------
        if do_attn and do_ctx:
            h = NH_LOC - 1
            # head 7's own chain: compact through the now-free "s" ring
            lastchain = ps_s("ps_l")
            for tk in range(8):
                mm2(lastchain, v_sb[tk][:, h, :], prev_probs[tk],
                    start=(tk == 0), stop=(tk == 7))
            # interleave the two final fins by query-half so the first
            # out-late units (columns 0-511) unblock after 4 DVE ops
            fin_part(h - 1, lastfin, 0, 2)
            fin_part(h, lastchain, 0, 2)
            fin_part(h - 1, lastfin, 1, 2)
            fin_part(h, lastchain, 1, 2)
        if do_out:
            for tc_i in range(8):
                unit_out_late(tc_i)

        if tok_d is not None:
            tk_t = small.tile([1, 4], F32, tag="tok", name="tok_t")
            nc.gpsimd.memset(tk_t[:], 0.0)
            nc.sync.dma_start(tok_d[:], tk_t[:])


def _get_nc():
    if "nc" not in _CACHE:
        _CACHE["nc"] = _build_graph()
    return _CACHE["nc"]


def kernel(x, mask, Wq, bq, Wk, bk, Wv, bv, Wo, bo):
    x = np.asarray(x, dtype=np.float32)
    Wq = np.asarray(Wq, dtype=np.float32)
    Wk = np.asarray(Wk, dtype=np.float32)
    Wv = np.asarray(Wv, dtype=np.float32)
    Wo = np.asarray(Wo, dtype=np.float32)

    nc = _get_nc()
    bf = ml_dtypes.bfloat16
    in_maps = []
    for c in range(8):
        b, g = c // 2, c % 2
        sl = slice(g * HG, (g + 1) * HG)
        in_maps.append(
            {
                "xt": np.ascontiguousarray(x[b].T.astype(bf)),
                "wqt": np.ascontiguousarray(Wq[sl, :].T.astype(bf)),
                "wkt": np.ascontiguousarray(Wk[sl, :].T.astype(bf)),
                "wvt": np.ascontiguousarray(Wv[sl, :].T.astype(bf)),
                "wot": np.ascontiguousarray(Wo[:, sl].T.astype(bf)),
                "ident": np.eye(P, dtype=bf),
            }
        )
    res = run_bass_kernel_spmd(
        nc, in_maps, core_ids=list(range(8)), **_CACHE.get("run_kwargs", {})
    )
    _CACHE["last_result"] = res
    outs = [res.results[c]["out_p"] for c in range(8)]
    return np.stack(
        [outs[2 * b] + outs[2 * b + 1] for b in range(4)]
    ).astype(np.float32)


# revision 15
# speedup vs baseline: 1.0018x; 1.0018x over previous
"""Distributed multi-head attention for trn2 (8 NeuronCores).

Problem: B=4, S=1024, H=1024, nh=16, hd=64; mask is all-ones, biases are
zero (both fixed by the problem's input spec), so neither reaches the
device.

Sharding: core c = b*2 + g handles batch b = c//2 and head-group
g = c%2 (8 heads = 512 hidden dims).  Each core computes
  qT,kT = (Wq_g @ x_b.T), (Wk_g @ x_b.T)      [512, 1024]
  v     = x_b @ Wv_g.T                          [1024, 512]
  per head: scoresT = kT_h.T-contract-qT_h      [tk, tq] psum
            probsT  = exp(scoresT / 8)          (ACT, fp8 out)
            ctxT_aug = [v_h | 1].T @ probsT     rows 0-63 ctx, row 64+ = rowsum
            ctxT = ctxT_aug[0:64] * (1/rowsum)  (DVE)
  partial_out = ctxT.T @ Wo_g_rows.T            [1024, 1024]
Host sums the two partials of each batch (row-parallel Wo unshard) and
stacks the 4 batches.

Schedule (v4): one interleaved PE stream.  The attention inner loop is
paced by ACT exp (~1147 ns per [128,1024] chunk) while its own PE work
is ~854 ns/slot, so all other matmul units ride the scores PSUM ring as
compact fillers: QK projections of pair m during heads 2m-2/2m-1, the V
projection in heads 0-1, and the first half of the out-projection
(cc0+cc1 -> bf16 partial) in heads 6-7.  The tail does the last head's
ctx (riding 2 slots behind its own exps), split fins, then per-column
units (cc2+cc3 [+ partial via identity matmul], evacuated alternately
by ACT copy and DVE add, DMA'd alternately via SP and GPSIMD queues).
PSUM: tag "s" bufs=2 + tag "c" bufs=2 = 8 banks.
"""

import sys

import numpy as np

sys.path.insert(0, "/opt/trn_rl_repo")

import ml_dtypes  # noqa: E402

import concourse.bass as bass  # noqa: E402
import concourse.tile as tile  # noqa: E402
from concourse import bacc, mybir  # noqa: E402
from concourse.bass_utils import run_bass_kernel_spmd  # noqa: E402

S = 1024  # sequence length
H = 1024  # hidden
NH_LOC = 8  # heads per core
HD = 64  # head dim
HG = 512  # hidden dims per core's head group
P = 128  # partitions

F32 = mybir.dt.float32
BF16 = mybir.dt.bfloat16
FP8E3 = mybir.dt.float8e3  # e3m4: 4 mantissa bits, ~0.9% probs quant
INPUT_DT = BF16

_CACHE: dict = {}

# schedule knobs (HW bisect)
USE_GPSIMD_DMA = False  # odd out-late DMAs via gpsimd SWDGE (else SP)
USE_ACT_QK_EVAC = False # qk psum evac split DVE+ACT (else 2x DVE)
USE_IDENT = False       # even out-late: ident-matmul + ACT copy (else DVE add)


def _build_graph(reps: int = 1, timing: bool = False, phases=("qkv", "attn", "exp", "ctx", "out")):
    nc = bacc.Bacc(
        "TRN2", target_bir_lowering=False, debug=False, num_devices=8
    )

    kind = "Internal" if timing else "ExternalInput"
    okind = "Internal" if timing else "ExternalOutput"
    xt_d = nc.dram_tensor("xt", [H, S], INPUT_DT, kind=kind).ap()
    wqt_d = nc.dram_tensor("wqt", [H, HG], INPUT_DT, kind=kind).ap()
    wkt_d = nc.dram_tensor("wkt", [H, HG], INPUT_DT, kind=kind).ap()
    wvt_d = nc.dram_tensor("wvt", [H, HG], INPUT_DT, kind=kind).ap()
    wot_d = nc.dram_tensor("wot", [HG, H], BF16, kind=kind).ap()
    id_d = nc.dram_tensor("ident", [P, P], BF16, kind=kind).ap()
    out_d = nc.dram_tensor("out_p", [S, H], F32, kind=okind).ap()
    tok_d = (
        nc.dram_tensor("tok", [1, 4], F32, kind="ExternalOutput").ap()
        if timing
        else None
    )

    with tile.TileContext(nc) as tc:
        with tc.tile_pool(name="inp", bufs=1) as inp:
            tiles = _dma_inputs(tc, inp, xt_d, wqt_d, wkt_d, wvt_d, wot_d, id_d)
            pre = _alloc_persistent(tc, inp)
            if reps == 1:
                _compute(tc, tiles, out_d, tok_d, phases, pre)
            else:
                hints = (
                    mybir.EngineType.PE,
                    mybir.EngineType.DVE,
                    mybir.EngineType.Pool,
                )
                with tc.For_i(0, reps, 1, hint_engines=hints):
                    _compute(tc, tiles, out_d, tok_d, phases, pre)

    nc.compile()
    return nc


def _dma_inputs(tc, inp, xt_d, wqt_d, wkt_d, wvt_d, wot_d, id_d):
    # interleave per contraction-chunk so the first qk accumulation
    # chains can start as soon as chunk 0 lands
    nc = tc.nc
    xt, wqt, wkt, wvt = [], [], [], []
    for kc in range(8):
        t = inp.tile([P, S], INPUT_DT, tag=f"xt{kc}", name=f"xt{kc}")
        nc.sync.dma_start(t[:], xt_d[kc * P : (kc + 1) * P, :])
        xt.append(t)
        for lst, d, tag in (
            (wqt, wqt_d, "wqt"), (wkt, wkt_d, "wkt"), (wvt, wvt_d, "wvt")
        ):
            t = inp.tile([P, HG], INPUT_DT, tag=f"{tag}{kc}", name=f"{tag}{kc}")
            nc.sync.dma_start(t[:], d[kc * P : (kc + 1) * P, :])
            lst.append(t)
    wot = []  # 4 x [128, 1024] bf16, rows = local c
    for cc in range(4):
        t = inp.tile([P, H], BF16, tag=f"wot{cc}", name=f"wot{cc}")
        nc.sync.dma_start(t[:], wot_d[cc * P : (cc + 1) * P, :])
        wot.append(t)
    ident = inp.tile([P, P], BF16, tag="ident", name="ident")
    nc.sync.dma_start(ident[:], id_d[:])
    return xt, wqt, wkt, wvt, wot, ident


def _alloc_persistent(tc, inp):
    """Persistent activation tiles + one-time constant fills (outside any
    For_i): rows 64-127 of qT/kT stay zero (K=128-padded scores); the
    ones columns of v_sb are never overwritten by the per-iteration
    copies, so both are filled once instead of every iteration."""
    nc = tc.nc
    pre = {}
    for h in range(NH_LOC):
        for tag in ("qT", "kT"):
            t = inp.tile([P, S], BF16, tag=f"{tag}{h}", name=f"{tag}{h}")
            nc.gpsimd.memset(t[64:128, :], 0.0)
            pre[f"{tag}{h}"] = t
    for i in range(8):
        # [key-in-chunk, head, 64 v-dims + 64 ones]
        t = inp.tile([P, 8, 128], BF16, tag=f"v{i}", name=f"v{i}")
        nc.gpsimd.memset(t[:, :, HD:128], 1.0)
        pre[f"v{i}"] = t
    for m in range(4):
        t = inp.tile([P, S], BF16, tag=f"ctxT{m}", name=f"ctxT{m}")
        pre[f"ctxT{m}"] = t
    return pre


def _compute(tc, tiles, out_d, tok_d=None, phases=("qkv", "attn", "exp", "ctx", "out"), pre=None):
    pre = pre or {}
    nc = tc.nc
    xt, wqt, wkt, wvt, wot, ident = tiles
    from contextlib import ExitStack

    do_qkv = "qkv" in phases
    do_attn = "attn" in phases
    do_exp = "exp" in phases
    do_ctx = "ctx" in phases
    do_out = "out" in phases

    ctx = ExitStack()
    with ctx:
        probs_pool = ctx.enter_context(tc.tile_pool(name="probs", bufs=18))
        small = ctx.enter_context(tc.tile_pool(name="small", bufs=3))
        outsb = ctx.enter_context(tc.tile_pool(name="outsb", bufs=1))
        # PSUM: tag "s" 3x[128,1024]f32 (6 banks) for scores/exp and every
        # compact filler unit; tag "c" 1x (2 banks) for the ctx chains
        psp = ctx.enter_context(tc.tile_pool(name="psp", bufs=3, space="PSUM"))

        def ps_s(name):
            return psp.tile([P, S], F32, tag="s", bufs=3, name=name)

        def ps_c(name):
            return psp.tile([P, S], F32, tag="c", bufs=1, name=name)

        def mm2(ps, lhsT, rhs, start, stop):
            # ISA limit: one matmul writes at most 512 f32 psum columns
            for th in range(2):
                nc.tensor.matmul(
                    ps[:, th * 512 : (th + 1) * 512],
                    lhsT,
                    rhs[:, th * 512 : (th + 1) * 512],
                    start=start,
                    stop=stop,
                )


        qT = [pre[f"qT{h}"] for h in range(NH_LOC)]
        kT = [pre[f"kT{h}"] for h in range(NH_LOC)]
        v_sb = [pre[f"v{i}"] for i in range(8)]
        ctxT = [pre[f"ctxT{m}"] for m in range(4)]
        partial = [
            outsb.tile([P, H], BF16, tag=f"pt{i}", name=f"pt{i}") for i in range(8)
        ]
        out_acc = [
            outsb.tile([P, H], F32, tag=f"oa{i}", name=f"oa{i}") for i in range(8)
        ]

        # ---- filler units (each rides the "s" PSUM ring, compact) -------
        def unit_qk(which, m):
            if not do_qkv:
                return
            ps = ps_s("ps_qk")
            w = wqt if which == "q" else wkt
            dst = qT if which == "q" else kT
            for kc in range(8):
                mm2(ps, w[kc][:, m * P : (m + 1) * P], xt[kc],
                    start=(kc == 0), stop=(kc == 7))
            # evacuate both head-halves in parallel: DVE + ACT copy so the
            # psum slot frees in ~1.2us instead of 2.4us of serial DVE
            nc.vector.tensor_copy(dst[2 * m][0:64, :], ps[0:64, :])
            if USE_ACT_QK_EVAC:
                nc.scalar.activation(
                    dst[2 * m + 1][0:64, :], ps[64:128, :],
                    mybir.ActivationFunctionType.Copy,
                )
            else:
                nc.vector.tensor_copy(dst[2 * m + 1][0:64, :], ps[64:128, :])

        def unit_v(tk):
            if not do_qkv:
                return
            ps = psp.tile([P, 8, HD], F32, tag="s", bufs=3, name="ps_v")
            for kc in range(8):
                nc.tensor.matmul(
                    ps[:],
                    xt[kc][:, tk * P : (tk + 1) * P],
                    wvt[kc][:],
                    start=(kc == 0),
                    stop=(kc == 7),
                )
            nc.vector.tensor_copy(v_sb[tk][:, :, 0:HD], ps[:])

        def unit_out_early(tc_i):
            if not do_out:
                return
            ps = ps_s("ps_oe")
            for cc in (0, 1):
                mm2(ps, ctxT[cc][:, tc_i * P : (tc_i + 1) * P], wot[cc],
                    start=(cc == 0), stop=(cc == 1))
            nc.vector.tensor_copy(partial[tc_i][:], ps[:])

        def unit_out_late(tc_i):
            if not do_out:
                return
            ps = ps_s("ps_ol")
            use_act = USE_IDENT and tc_i % 2 == 0
            mm2(ps, ctxT[2][:, tc_i * P : (tc_i + 1) * P], wot[2],
                start=True, stop=False)
            mm2(ps, ctxT[3][:, tc_i * P : (tc_i + 1) * P], wot[3],
                start=False, stop=not use_act)
            if use_act:
                # fold the bf16 partial into psum via identity matmul, then
                # ACT evacuates; keeps DVE free for the odd units' adds
                mm2(ps, ident[:], partial[tc_i], start=False, stop=True)
                nc.scalar.activation(
                    out_acc[tc_i][:], ps[:],
                    mybir.ActivationFunctionType.Copy,
                )
                eng = nc.sync
            else:
                nc.vector.tensor_tensor(
                    out_acc[tc_i][:], ps[:], partial[tc_i][:],
                    mybir.AluOpType.add,
                )
                eng = nc.gpsimd if USE_GPSIMD_DMA else nc.sync
            eng.dma_start(
                out_d[tc_i * P : (tc_i + 1) * P, :], out_acc[tc_i][:]
            )

        def fin_part(h, pchain, j, nhalf):
            if not do_ctx:
                return
            m, hh = h // 2, h % 2
            w = S // nhalf
            sl = slice(j * w, (j + 1) * w)
            rp = small.tile([HD, w], F32, tag="rp", name="rp")
            nc.vector.reciprocal(rp[:], pchain[64:128, sl])
            nc.vector.tensor_tensor(
                ctxT[m][hh * HD : (hh + 1) * HD, sl],
                pchain[0:HD, sl],
                rp[:],
                mybir.AluOpType.mult,
            )

        def fin(h, pchain):
            fin_part(h, pchain, 0, 1)

        # filler placement: (head, slot) -> unit thunks.  One compact
        # unit per ~2 slots in the low-PE half (slots 0-3) of each head.
        # Deadlines: v pair (2j,2j+1) before ctx(0) slot 4+j in head 1;
        # qk pair m before head 2m slot 0; out-early after fin(3) (head
        # 4/5 boundary).
        fillers = {}

        def put(h, s, u):
            fillers.setdefault((h, s), []).append(u)

        put(0, 0, lambda: unit_qk("q", 1))
        put(0, 2, lambda: unit_v(0))
        put(0, 4, lambda: unit_v(1))
        put(0, 6, lambda: unit_v(2))
        put(1, 0, lambda: unit_qk("k", 1))
        put(1, 1, lambda: unit_v(3))
        put(1, 2, lambda: unit_v(4))
        put(1, 3, lambda: unit_v(5))
        put(1, 4, lambda: unit_v(6))
        put(1, 5, lambda: unit_v(7))
        put(2, 0, lambda: unit_qk("q", 2))
        put(3, 0, lambda: unit_qk("k", 2))
        put(4, 0, lambda: unit_qk("q", 3))
        put(5, 0, lambda: unit_qk("k", 3))
        for i in range(4):
            put(6, i, lambda i=i: unit_out_early(i))
            put(7, i, lambda i=i: unit_out_early(4 + i))

        # ---- prologue: pair 0 projections -------------------------------
        unit_qk("q", 0)
        unit_qk("k", 0)

        # ---- attention loop --------------------------------------------
        # ctx chain of head h-1 runs compressed in head h slots 4-7 (two
        # accumulation matmuls per slot; all its probs are ready), fin
        # right after -> the single "c" psum tile frees early in head h+1,
        # well before the next chain allocates at head h+1 slot 4.
        prev_probs = None
        for h in range(NH_LOC):
            if not do_attn:
                break
            newchain = None
            last = h == NH_LOC - 1
            probs = []
            for tk in range(8):
                ps = ps_s("ps_s")
                mm2(ps, kT[h][:, tk * P : (tk + 1) * P], qT[h],
                    start=True, stop=True)
                pb = probs_pool.tile([P, S], FP8E3, tag="pb", name="pb")
                if do_exp:
                    nc.scalar.activation(
                        pb[:], ps[:], mybir.ActivationFunctionType.Exp,
                        scale=0.125,
                    )
                probs.append(pb)
                if do_ctx and prev_probs is not None and tk >= 4:
                    if newchain is None:
                        newchain = ps_c("ps_c")
                    for j in (0, 1):
                        ck = 2 * (tk - 4) + j
                        mm2(newchain, v_sb[ck][:, h - 1, :], prev_probs[ck],
                            start=(ck == 0), stop=(ck == 7))
                for u in fillers.get((h, tk), []):
                    u()
            if newchain is not None and not last:
                fin(h - 1, newchain)
            prev_probs = probs
            if last:
                lastfin = newchain  # chain of head 6, finned in the tail

        # ---- tail -------------------------------------------------------
        if do_attn and do_ctx:
            h = NH_LOC - 1
            # head 7's own chain: compact through the now-free "s" ring
            lastchain = ps_s("ps_l")
            for tk in range(8):
                mm2(lastchain, v_sb[tk][:, h, :], prev_probs[tk],
                    start=(tk == 0), stop=(tk == 7))
            # interleave the two final fins by query-half so the first
            # out-late units (columns 0-511) unblock after 4 DVE ops
            fin_part(h - 1, lastfin, 0, 2)
            fin_part(h, lastchain, 0, 2)
            fin_part(h - 1, lastfin, 1, 2)
            fin_part(h, lastchain, 1, 2)
        if do_out:
            for tc_i in range(8):
                unit_out_late(tc_i)

        if tok_d is not None:
            tk_t = small.tile([1, 4], F32, tag="tok", name="tok_t")
            nc.gpsimd.memset(tk_t[:], 0.0)
            nc.sync.dma_start(tok_d[:], tk_t[:])


def _get_nc():
    if "nc" not in _CACHE:
        _CACHE["nc"] = _build_graph()
    return _CACHE["nc"]


def kernel(x, mask, Wq, bq, Wk, bk, Wv, bv, Wo, bo):
    x = np.asarray(x, dtype=np.float32)
    Wq = np.asarray(Wq, dtype=np.float32)
    Wk = np.asarray(Wk, dtype=np.float32)
    Wv = np.asarray(Wv, dtype=np.float32)
    Wo = np.asarray(Wo, dtype=np.float32)

    nc = _get_nc()
    bf = ml_dtypes.bfloat16
    in_maps = []
    for c in range(8):
        b, g = c // 2, c % 2
        sl = slice(g * HG, (g + 1) * HG)
        in_maps.append(
            {
                "xt": np.ascontiguousarray(x[b].T.astype(bf)),
                "wqt": np.ascontiguousarray(Wq[sl, :].T.astype(bf)),
                "wkt": np.ascontiguousarray(Wk[sl, :].T.astype(bf)),
                "wvt": np.ascontiguousarray(Wv[sl, :].T.astype(bf)),
                "wot": np.ascontiguousarray(Wo[:, sl].T.astype(bf)),
                "ident": np.eye(P, dtype=bf),
            }
        )
    res = run_bass_kernel_spmd(
        nc, in_maps, core_ids=list(range(8)), **_CACHE.get("run_kwargs", {})
    )
    _CACHE["last_result"] = res
    outs = [res.results[c]["out_p"] for c in range(8)]
    return np.stack(
        [outs[2 * b] + outs[2 * b + 1] for b in range(4)]
    ).astype(np.float32)


# revision 16
# speedup vs baseline: 1.1964x; 1.1943x over previous
"""Distributed multi-head attention for trn2 (8 NeuronCores).

Problem: B=4, S=1024, H=1024, nh=16, hd=64; mask is all-ones, biases are
zero (both fixed by the problem's input spec), so neither reaches the
device.

Sharding: core c = b*2 + g handles batch b = c//2 and head-group
g = c%2 (8 heads = 512 hidden dims).  Each core computes
  qT,kT = (Wq_g @ x_b.T), (Wk_g @ x_b.T)      [512, 1024]  (f32r matmuls)
  v     = x_b @ Wv_g.T                          [1024, 512]
  per head: scoresT = kT_h.T-contract-qT_h      [tk, tq] psum
            probsT  = exp(scoresT / 8)          (ACT, bf16 out)
            ctxT_aug = [v_h | 1].T @ probsT     rows 0-63 ctx, row 64 = rowsum
            ctxT = ctxT_aug[0:64] * (1/rowsum)  (DVE)
  partial_out = ctxT.T @ Wo_g_rows.T            [1024, 1024]  (f32r)
Host sums the two partials of each batch (row-parallel Wo unshard) and
stacks the 4 batches.
"""

import sys

import numpy as np

sys.path.insert(0, "/opt/trn_rl_repo")

import ml_dtypes  # noqa: E402

import concourse.bass as bass  # noqa: E402
import concourse.tile as tile  # noqa: E402
from concourse import bacc, mybir  # noqa: E402
from concourse.bass_utils import run_bass_kernel_spmd  # noqa: E402

S = 1024  # sequence length
H = 1024  # hidden
NH_LOC = 8  # heads per core
HD = 64  # head dim
HG = 512  # hidden dims per core's head group
P = 128  # partitions

F32 = mybir.dt.float32
F32R = mybir.dt.float32r
BF16 = mybir.dt.bfloat16
FP8E3 = mybir.dt.float8e3  # e3m4: 4 mantissa bits, ~1.3% probs quant (fits 2e-2 budget)
INPUT_DT = BF16  # bf16 end-to-end: K=128 full-rate matmuls, FWL loads

_CACHE: dict = {}


def CTX_DT():
    return BF16


def _build_graph(reps: int = 1, timing: bool = False, phases=("qkv", "attn", "exp", "ctx", "out")):
    nc = bacc.Bacc(
        "TRN2", target_bir_lowering=False, debug=False, num_devices=8
    )

    kind = "Internal" if timing else "ExternalInput"
    okind = "Internal" if timing else "ExternalOutput"
    xt_d = nc.dram_tensor("xt", [H, S], INPUT_DT, kind=kind).ap()
    wqt_d = nc.dram_tensor("wqt", [H, HG], INPUT_DT, kind=kind).ap()
    wkt_d = nc.dram_tensor("wkt", [H, HG], INPUT_DT, kind=kind).ap()
    wvt_d = nc.dram_tensor("wvt", [H, HG], INPUT_DT, kind=kind).ap()
    wot_d = nc.dram_tensor("wot", [HG, H], BF16, kind=kind).ap()
    out_d = nc.dram_tensor("out_p", [S, H], F32, kind=okind).ap()
    tok_d = (
        nc.dram_tensor("tok", [1, 4], F32, kind="ExternalOutput").ap()
        if timing
        else None
    )

    with tile.TileContext(nc) as tc:
        if reps == 1:
            _body(tc, xt_d, wqt_d, wkt_d, wvt_d, wot_d, out_d, tok_d)
        else:
            # timing loop: load inputs once, loop the compute body so the
            # per-iteration time is the steady-state compute pipeline
            with tc.tile_pool(name="inp", bufs=1) as inp:
                tiles = _dma_inputs(tc, inp, xt_d, wqt_d, wkt_d, wvt_d, wot_d)
                nc = tc.nc
                pre = _alloc_persistent(tc, inp)
                if "qkv" not in phases:
                    if "attn" in phases:
                        for tag in ("qT", "kT"):
                            for h in range(NH_LOC):
                                nc.gpsimd.memset(pre[f"{tag}{h}"][:], 0.125)
                    if "ctx" in phases:
                        for tc_i in range(8):
                            nc.gpsimd.memset(pre[f"v{tc_i}"][:], 0.125)
                if "out" in phases and "ctx" not in phases:
                    for i in range(4):
                        nc.gpsimd.memset(pre[f"ctxT{i}"][:], 0.125)
                hints = (
                    mybir.EngineType.PE,
                    mybir.EngineType.DVE,
                    mybir.EngineType.Pool,
                )
                with tc.For_i(0, reps, 1, hint_engines=hints):
                    _compute(tc, tiles, out_d, tok_d, phases, pre)

    nc.compile()
    return nc


def _dma_inputs(tc, inp, xt_d, wqt_d, wkt_d, wvt_d, wot_d):
    # interleave per contraction-chunk so the first qk/v accumulation
    # chains can start as soon as chunk 0 lands instead of waiting for
    # whole tensors
    nc = tc.nc
    xt, wqt, wkt, wvt = [], [], [], []
    for kc in range(8):
        t = inp.tile([P, S], INPUT_DT, tag=f"xt{kc}", name=f"xt{kc}")
        nc.sync.dma_start(t[:], xt_d[kc * P : (kc + 1) * P, :])
        xt.append(t)
        for lst, d, tag in (
            (wqt, wqt_d, "wqt"), (wkt, wkt_d, "wkt"), (wvt, wvt_d, "wvt")
        ):
            t = inp.tile([P, HG], INPUT_DT, tag=f"{tag}{kc}", name=f"{tag}{kc}")
            nc.sync.dma_start(t[:], d[kc * P : (kc + 1) * P, :])
            lst.append(t)
    wot = []  # 4 x [128, 1024] bf16, rows = local c
    for cc in range(4):
        t = inp.tile([P, H], BF16, tag=f"wot{cc}", name=f"wot{cc}")
        nc.sync.dma_start(t[:], wot_d[cc * P : (cc + 1) * P, :])
        wot.append(t)
    return xt, wqt, wkt, wvt, wot


def _body(tc, xt_d, wqt_d, wkt_d, wvt_d, wot_d, out_d, tok_d=None):
    with tc.tile_pool(name="inp", bufs=1) as inp:
        tiles = _dma_inputs(tc, inp, xt_d, wqt_d, wkt_d, wvt_d, wot_d)
        pre = _alloc_persistent(tc, inp)
        _compute(tc, tiles, out_d, tok_d, pre=pre)


def _alloc_persistent(tc, inp):
    """Persistent activation tiles + one-time constant fills (outside any
    For_i): rows 64-127 of qT/kT stay zero (K=128-padded scores); the
    ones columns of v_sb are never overwritten by the per-iteration
    copies, so both are filled once instead of every iteration."""
    nc = tc.nc
    pre = {}
    for h in range(NH_LOC):
        for tag in ("qT", "kT"):
            t = inp.tile([P, S], BF16, tag=f"{tag}{h}", name=f"{tag}{h}")
            nc.gpsimd.memset(t[64:128, :], 0.0)
            pre[f"{tag}{h}"] = t
    for i in range(8):
        # [key-in-chunk, head, 64 v-dims + 64 ones]
        t = inp.tile([P, 8, 128], BF16, tag=f"v{i}", name=f"v{i}")
        nc.gpsimd.memset(t[:, :, HD:128], 1.0)
        pre[f"v{i}"] = t
    for m in range(4):
        t = inp.tile([P, S], CTX_DT(), tag=f"ctxT{m}", name=f"ctxT{m}")
        pre[f"ctxT{m}"] = t
    return pre


def _compute(tc, tiles, out_d, tok_d=None, phases=("qkv", "attn", "exp", "ctx", "out"), pre=None):
    pre = pre or {}
    nc = tc.nc
    xt, wqt, wkt, wvt, wot = tiles
    from contextlib import ExitStack

    ctx = ExitStack()
    with ctx:
        acts = ctx.enter_context(tc.tile_pool(name="acts", bufs=1))
        probs_pool = ctx.enter_context(tc.tile_pool(name="probs", bufs=20))
        small = ctx.enter_context(tc.tile_pool(name="small", bufs=3))
        outsb = ctx.enter_context(tc.tile_pool(name="outsb", bufs=1))
        ps_big = ctx.enter_context(
            tc.tile_pool(name="ps_big", bufs=2, space="PSUM")
        )
        ps_sm = ctx.enter_context(
            tc.tile_pool(name="ps_sm", bufs=4, space="PSUM")
        )

        do_qkv = "qkv" in phases
        do_attn = "attn" in phases
        do_exp = "exp" in phases
        do_ctx = "ctx" in phases
        do_out = "out" in phases

        # ---- persistent tiles ------------------------------------------
        # qT/kT: one [128, 1024] bf16 tile PER HEAD; rows 0-63 = that
        # head's [d, t], rows 64-127 = 0 so the scores matmul contracts
        # K=128 (K=64 streams at half rate on the PE)
        qT = [
            pre.get(f"qT{h}")
            or acts.tile([P, S], BF16, tag=f"qT{h}", name=f"qT{h}")
            for h in range(NH_LOC)
        ]
        kT = [
            pre.get(f"kT{h}")
            or acts.tile([P, S], BF16, tag=f"kT{h}", name=f"kT{h}")
            for h in range(NH_LOC)
        ]
        # v_aug: 8 tiles [128, 8 heads, 64 v + 64 ones] bf16 (64
        # ones-columns put 64 copies of the softmax denominator on psum
        # rows 64-127 - no broadcast needed); ones filled once in
        # _alloc_persistent
        v_sb = [
            pre.get(f"v{i}")
            or acts.tile([P, 8, 128], BF16, tag=f"v{i}", name=f"v{i}")
            for i in range(8)
        ]
        # ctxT: 4 tiles [128, 1024] bf16 (outproj stationary); tile m =
        # heads 2m, 2m+1 (partition = local c dim)
        ctxT = [
            pre.get(f"ctxT{i}")
            or acts.tile([P, S], CTX_DT(), tag=f"ctxT{i}", name=f"ctxT{i}")
            for i in range(4)
        ]
        # out accumulators: 8 tiles [128, 1024] f32
        out_acc = [
            outsb.tile([P, H], F32, tag=f"oa{i}", name=f"oa{i}") for i in range(8)
        ]

        def emit_qk(m):
            for w, lst in ((wqt, qT), (wkt, kT)):
                ps = ps_big.tile([P, S], F32, tag="ps", name="ps_qk")
                for kc in range(8):
                    for th in range(2):
                        nc.tensor.matmul(
                            ps[:, th * 512 : (th + 1) * 512],
                            w[kc][:, m * P : (m + 1) * P],
                            xt[kc][:, th * 512 : (th + 1) * 512],
                            start=(kc == 0),
                            stop=(kc == 7),
                        )
                nc.vector.tensor_copy(lst[2 * m][0:64, :], ps[0:64, :])
                nc.vector.tensor_copy(lst[2 * m + 1][0:64, :], ps[64:128, :])

        def emit_ctx_mm(h, tk, ps_cs, probs):
            for th in range(2):
                nc.tensor.matmul(
                    ps_cs[th][:],
                    v_sb[tk][:, h, :],
                    probs[tk][:, th * 512 : (th + 1) * 512],
                    start=(tk == 0),
                    stop=(tk == 7),
                )

        def emit_ctx_fin(h, ps_cs):
            m, hh = h // 2, h % 2
            for th in range(2):
                rp = small.tile([HD, 512], F32, tag="recip", name="rp")
                nc.vector.reciprocal(rp[:], ps_cs[th][64:128, :])
                nc.vector.tensor_tensor(
                    ctxT[m][hh * HD : (hh + 1) * HD, th * 512 : (th + 1) * 512],
                    ps_cs[th][0:HD, :],
                    rp[:],
                    mybir.AluOpType.mult,
                )

        def emit_out_all():
            for tc_i in range(8):
                for ho in range(2):
                    ps = ps_sm.tile([P, 512], F32, tag="ps", name="ps_o")
                    for cc in range(4):
                        nc.tensor.matmul(
                            ps[:],
                            ctxT[cc][:, tc_i * P : (tc_i + 1) * P],
                            wot[cc][:, ho * 512 : (ho + 1) * 512],
                            start=(cc == 0),
                            stop=(cc == 3),
                        )
                    nc.scalar.activation(
                        out_acc[tc_i][:, ho * 512 : (ho + 1) * 512], ps[:],
                        mybir.ActivationFunctionType.Copy,
                    )
                nc.sync.dma_start(
                    out_d[tc_i * P : (tc_i + 1) * P, :], out_acc[tc_i][:]
                )

        # ---- attention: per head-pair m, interleaved ctx one head back --
        if do_qkv:
            for m in range(4):
                emit_qk(m)

        # ---- V projection ----------------------------------------------
        def emit_v(tc_i):
            ps = ps_sm.tile([P, 8, HD], F32, tag="ps", name="ps_v")
            for kc in range(8):
                nc.tensor.matmul(
                    ps[:],
                    xt[kc][:, tc_i * P : (tc_i + 1) * P],
                    wvt[kc][:],
                    start=(kc == 0),
                    stop=(kc == 7),
                )
            # single strided copy: psum [128, (8 heads, 64 dims)] ->
            # v_sb dim columns (ones columns untouched)
            nc.vector.tensor_copy(v_sb[tc_i][:, :, 0:HD], ps[:])

        if do_qkv and not do_attn:
            for tc_i in range(8):
                emit_v(tc_i)

        prev = None  # probs of head h-1
        pchain = None
        for h in range(NH_LOC):
            if not do_attn:
                break
            pchain = (
                [
                    ps_sm.tile([P, 512], F32, tag="ps", name="ps_c")
                    for _ in range(2)
                ]
                if (do_ctx and prev is not None)
                else None
            )
            probs = []
            for tk in range(8):
                ps = ps_big.tile([P, S], F32, tag="ps", name="ps_s")
                for th in range(2):
                    nc.tensor.matmul(
                        ps[:, th * 512 : (th + 1) * 512],
                        kT[h][:, tk * P : (tk + 1) * P],
                        qT[h][:, th * 512 : (th + 1) * 512],
                        start=True,
                        stop=True,
                    )
                pb = probs_pool.tile([P, S], FP8E3, tag="probs", name="pb")
                if do_exp:
                    nc.scalar.activation(
                        pb[:], ps[:], mybir.ActivationFunctionType.Exp,
                        scale=0.125,
                    )
                probs.append(pb)
                # scores/exp lead the slot so ACT's input is ready
                # earliest; the lagging ctx chain and fillers ride behind
                if pchain is not None:
                    emit_ctx_mm(h - 1, tk, pchain, prev)
                # V-proj rides in head 0's slots (PE-light: no lagging
                # ctx chain yet) and finishes each v_sb[tk] exactly one
                # head before ctx_0 reads it
                if h == 0 and do_qkv:
                    emit_v(tk)
            if pchain is not None:
                emit_ctx_fin(h - 1, pchain)
            prev = probs
        if do_attn and do_ctx:
            h = NH_LOC - 1
            pchain = [
                ps_sm.tile([P, 512], F32, tag="ps", name="ps_c") for _ in range(2)
            ]
            for tk in range(8):
                emit_ctx_mm(h, tk, pchain, prev)
            emit_ctx_fin(h, pchain)
        if do_out:
            emit_out_all()

        if tok_d is not None:
            tk_t = small.tile([1, 4], F32, tag="tok")
            nc.gpsimd.memset(tk_t[:], 0.0)
            nc.sync.dma_start(tok_d[:], tk_t[:])


def _get_nc():
    if "nc" not in _CACHE:
        _CACHE["nc"] = _build_graph()
    return _CACHE["nc"]


def kernel(x, mask, Wq, bq, Wk, bk, Wv, bv, Wo, bo):
    x = np.asarray(x, dtype=np.float32)
    Wq = np.asarray(Wq, dtype=np.float32)
    Wk = np.asarray(Wk, dtype=np.float32)
    Wv = np.asarray(Wv, dtype=np.float32)
    Wo = np.asarray(Wo, dtype=np.float32)

    nc = _get_nc()
    bf = ml_dtypes.bfloat16 if INPUT_DT == BF16 else np.float32
    in_maps = []
    for c in range(8):
        b, g = c // 2, c % 2
        sl = slice(g * HG, (g + 1) * HG)
        in_maps.append(
            {
                "xt": np.ascontiguousarray(x[b].T.astype(bf)),
                "wqt": np.ascontiguousarray(Wq[sl, :].T.astype(bf)),
                "wkt": np.ascontiguousarray(Wk[sl, :].T.astype(bf)),
                "wvt": np.ascontiguousarray(Wv[sl, :].T.astype(bf)),
                "wot": np.ascontiguousarray(Wo[:, sl].T.astype(ml_dtypes.bfloat16)),
            }
        )
    res = run_bass_kernel_spmd(
        nc, in_maps, core_ids=list(range(8)), **_CACHE.get("run_kwargs", {})
    )
    _CACHE["last_result"] = res
    outs = [res.results[c]["out_p"] for c in range(8)]
    return np.stack(
        [outs[2 * b] + outs[2 * b + 1] for b in range(4)]
    ).astype(np.float32)

